# revision 6
# baseline (speedup 1.0000x reference)
"""Trainium2 Bass kernel for nn_LocalClassifier (moe_routing).

Computation (reference):
    xr     = x.reshape(B, P, F)            # [32, 784, 2048] fp32
    Wg     = W[target]                     # [32, 2048]  per-batch gathered row
    logits = einsum('bpf,bf->bp', xr, Wg) + b[target][:, None]
    out    = sigmoid(logits).reshape(-1, 1, 1, 1)    # [25088, 1, 1, 1]

Strategy (8 NeuronCores, data parallel over B):
  - Host gathers the 4 W rows / bias values each core needs (the "routing"),
    shards B across the 8 cores (4 batches -> 3136 of the 25088 rows each),
    and pre-transposes each core's x shard to feature-major layout
    [batch, partition=f%128, chunk=f//128, pixel] so the TensorEngine can
    contract over features (K on partitions) with plain contiguous DMAs.
  - Device streams 4 x-tiles of [128, 16*784] fp32 (6.4 MB DMAs), runs
    128x1x392 matmuls accumulating the 16 feature chunks into PSUM, then a
    fused bias+sigmoid on the ScalarEngine, and writes the 3136 outputs.
  - Memory-bound: 25.7 MB/core HBM reads at ~360 GB/s -> ~72 us roofline.
"""

import sys

sys.path.insert(0, "/opt/trn_rl_repo")

import numpy as np

import concourse.bacc as bacc
import concourse.mybir as mybir
import concourse.tile as tile
from concourse.bass_utils import run_bass_kernel_spmd

B = 32      # batches
P = 784     # pixels per batch
F = 2048    # features
NCORES = 8
BPC = B // NCORES          # 4 batches per core
KC = F // 128              # 16 feature chunks of 128
NH = 2                     # split P into 2 matmul halves (PSUM bank = 512 fp32)
NHALF = P // NH            # 392

FP32 = mybir.dt.float32

_NC_CACHE = {}


def _build_nc():
    nc = bacc.Bacc()
    xt = nc.declare_dram_parameter("xt", [BPC, 128, KC * P], FP32, isOutput=False)
    wg = nc.declare_dram_parameter("wg", [128, BPC * KC], FP32, isOutput=False)
    bg = nc.declare_dram_parameter("bg", [1, BPC], FP32, isOutput=False)
    out = nc.declare_dram_parameter("out", [1, BPC * P], FP32, isOutput=True)

    with tile.TileContext(nc) as tc:
        with (
            tc.tile_pool(name="xpool", bufs=3) as xpool,
            tc.tile_pool(name="cpool", bufs=1) as cpool,
            tc.tile_pool(name="psum", bufs=4, space="PSUM") as pp,
        ):
            wg_sb = cpool.tile([128, BPC * KC], FP32)
            nc.sync.dma_start(out=wg_sb[:], in_=wg[:])
            bg_sb = cpool.tile([1, BPC], FP32)
            nc.sync.dma_start(out=bg_sb[:], in_=bg[:])
            out_sb = cpool.tile([1, BPC * P], FP32)

            for b in range(BPC):
                xt_sb = xpool.tile([128, KC * P], FP32)
                # alternate the two HWDGE rings (SP / ACT) for the big loads
                eng = nc.sync if b % 2 == 0 else nc.scalar
                eng.dma_start(out=xt_sb[:], in_=xt[b])

                ps = [
                    pp.tile([1, NHALF], FP32, name=f"ps{h}", tag=f"ps{h}")
                    for h in range(NH)
                ]
                for k in range(KC):
                    col = b * KC + k
                    for h in range(NH):
                        nc.tensor.matmul(
                            ps[h][:, :],
                            wg_sb[:, col : col + 1],
                            xt_sb[:, k * P + h * NHALF : k * P + (h + 1) * NHALF],
                            start=(k == 0),
                            stop=(k == KC - 1),
                        )
                for h in range(NH):
                    nc.scalar.activation(
                        out_sb[0:1, b * P + h * NHALF : b * P + (h + 1) * NHALF],
                        ps[h][:, :],
                        mybir.ActivationFunctionType.Sigmoid,
                        bias=bg_sb[0:1, b : b + 1],
                        scale=1.0,
                    )

            nc.sync.dma_start(out=out[:], in_=out_sb[:])

    nc.finalize()
    return nc


def _get_nc():
    if "nc" not in _NC_CACHE:
        _NC_CACHE["nc"] = _build_nc()
    return _NC_CACHE["nc"]


def _make_in_maps(x, target, W, b):
    x = np.asarray(x, dtype=np.float32).reshape(B, P, F)
    target = np.asarray(target).astype(np.int64)
    W = np.asarray(W, dtype=np.float32)
    b = np.asarray(b, dtype=np.float32)

    Wg = W[target]          # [B, F]
    bg = b[target]          # [B]

    in_maps = []
    for m in range(NCORES):
        sl = slice(m * BPC, (m + 1) * BPC)
        # [BPC, P, KC, 128] -> [BPC, 128(part), KC, P] feature-major
        xs = x[sl].reshape(BPC, P, KC, 128).transpose(0, 3, 2, 1)
        xt = np.ascontiguousarray(xs).reshape(BPC, 128, KC * P)
        # wg[p, b*KC + k] = Wg[b, k*128 + p]
        wgc = Wg[sl].reshape(BPC, KC, 128).transpose(2, 0, 1).reshape(128, BPC * KC)
        in_maps.append(
            {
                "xt": xt,
                "wg": np.ascontiguousarray(wgc),
                "bg": bg[sl].reshape(1, BPC).copy(),
            }
        )
    return in_maps


def run(x, target, W, b, trace=False, **trace_kwargs):
    """Run on 8 cores; returns (full_output, BassKernelResults)."""
    nc = _get_nc()
    in_maps = _make_in_maps(x, target, W, b)
    res = run_bass_kernel_spmd(
        nc, in_maps, list(range(NCORES)), trace=trace, **trace_kwargs
    )
    outs = [res.results[i]["out"][0] for i in range(NCORES)]
    full = np.concatenate(outs, axis=0).reshape(-1, 1, 1, 1).astype(np.float32)
    return full, res


def kernel(x, target, W, b):
    full, _ = run(x, target, W, b, trace=False)
    return full


# revision 7
# speedup vs baseline: 2.7011x; 2.7011x over previous
"""Trainium2 Bass kernel for nn_LocalClassifier (moe_routing).

Computation (reference):
    xr     = x.reshape(B, P, F)            # [32, 784, 2048] fp32
    Wg     = W[target]                     # [32, 2048]  per-batch gathered row
    logits = einsum('bpf,bf->bp', xr, Wg) + b[target][:, None]
    out    = sigmoid(logits).reshape(-1, 1, 1, 1)    # [25088, 1, 1, 1]

Strategy (8 NeuronCores, data parallel over B):
  - Host gathers the 4 W rows / bias values each core needs (the "routing"),
    shards B across the 8 cores (4 batches -> 3136 of the 25088 rows each),
    and pre-transposes each core's x shard to feature-major layout
    [batch, partition=f%128, chunk=f//128, pixel] so the TensorEngine can
    contract over features (K on partitions) with plain contiguous DMAs.
  - Device streams 4 x-tiles of [128, 16*784] fp32 (6.4 MB DMAs), runs
    128x1x392 matmuls accumulating the 16 feature chunks into PSUM, then a
    fused bias+sigmoid on the ScalarEngine, and writes the 3136 outputs.
  - Memory-bound: 25.7 MB/core HBM reads at ~360 GB/s -> ~72 us roofline.
"""

import sys

sys.path.insert(0, "/opt/trn_rl_repo")

import numpy as np

import concourse.bacc as bacc
import concourse.mybir as mybir
import concourse.tile as tile
from concourse.bass_utils import run_bass_kernel_spmd

B = 32      # batches
P = 784     # pixels per batch
F = 2048    # features
NCORES = 8
BPC = B // NCORES          # 4 batches per core
KC = F // 128              # 16 feature chunks of 128
NH = 2                     # split P into 2 matmul halves (PSUM bank = 512 fp32)
NHALF = P // NH            # 392

FP32 = mybir.dt.float32
FP16 = mybir.dt.float16

_NC_CACHE = {}


def _build_nc():
    nc = bacc.Bacc()
    xt = nc.declare_dram_parameter("xt", [BPC, 128, KC * P], FP16, isOutput=False)
    wg = nc.declare_dram_parameter("wg", [128, BPC * KC], FP16, isOutput=False)
    bg = nc.declare_dram_parameter("bg", [1, BPC], FP32, isOutput=False)
    out = nc.declare_dram_parameter("out", [1, BPC * P], FP32, isOutput=True)

    with tile.TileContext(nc) as tc:
        with (
            tc.tile_pool(name="xpool", bufs=3) as xpool,
            tc.tile_pool(name="cpool", bufs=1) as cpool,
            tc.tile_pool(name="psum", bufs=4, space="PSUM") as pp,
        ):
            wg_sb = cpool.tile([128, BPC * KC], FP16)
            nc.sync.dma_start(out=wg_sb[:], in_=wg[:])
            bg_sb = cpool.tile([1, BPC], FP32)
            nc.sync.dma_start(out=bg_sb[:], in_=bg[:])
            out_sb = cpool.tile([1, BPC * P], FP32)

            for b in range(BPC):
                xt_sb = xpool.tile([128, KC * P], FP16)
                # keep every big load on the SP HWDGE ring: a second ring
                # round-robins at packet granularity and doubles the
                # completion latency of the first tile
                nc.sync.dma_start(out=xt_sb[:], in_=xt[b])

                ps = [
                    pp.tile([1, NHALF], FP32, name=f"ps{h}", tag=f"ps{h}")
                    for h in range(NH)
                ]
                for k in range(KC):
                    col = b * KC + k
                    for h in range(NH):
                        nc.tensor.matmul(
                            ps[h][:, :],
                            wg_sb[:, col : col + 1],
                            xt_sb[:, k * P + h * NHALF : k * P + (h + 1) * NHALF],
                            start=(k == 0),
                            stop=(k == KC - 1),
                        )
                for h in range(NH):
                    nc.scalar.activation(
                        out_sb[0:1, b * P + h * NHALF : b * P + (h + 1) * NHALF],
                        ps[h][:, :],
                        mybir.ActivationFunctionType.Sigmoid,
                        bias=bg_sb[0:1, b : b + 1],
                        scale=1.0,
                    )

            nc.sync.dma_start(out=out[:], in_=out_sb[:])

    nc.finalize()
    return nc


def _get_nc():
    if "nc" not in _NC_CACHE:
        _NC_CACHE["nc"] = _build_nc()
    return _NC_CACHE["nc"]


def _make_in_maps(x, target, W, b):
    x = np.asarray(x, dtype=np.float32).reshape(B, P, F)
    target = np.asarray(target).astype(np.int64)
    W = np.asarray(W, dtype=np.float32)
    b = np.asarray(b, dtype=np.float32)

    Wg = W[target]          # [B, F]
    bg = b[target]          # [B]

    in_maps = []
    for m in range(NCORES):
        sl = slice(m * BPC, (m + 1) * BPC)
        # [BPC, P, KC, 128] -> [BPC, 128(part), KC, P] feature-major
        xs = x[sl].reshape(BPC, P, KC, 128).transpose(0, 3, 2, 1)
        xt = np.ascontiguousarray(xs).reshape(BPC, 128, KC * P).astype(np.float16)
        # wg[p, b*KC + k] = Wg[b, k*128 + p]
        wgc = Wg[sl].reshape(BPC, KC, 128).transpose(2, 0, 1).reshape(128, BPC * KC).astype(np.float16)
        in_maps.append(
            {
                "xt": xt,
                "wg": np.ascontiguousarray(wgc),
                "bg": bg[sl].reshape(1, BPC).copy(),
            }
        )
    return in_maps


def run(x, target, W, b, trace=False, **trace_kwargs):
    """Run on 8 cores; returns (full_output, BassKernelResults)."""
    nc = _get_nc()
    in_maps = _make_in_maps(x, target, W, b)
    res = run_bass_kernel_spmd(
        nc, in_maps, list(range(NCORES)), trace=trace, **trace_kwargs
    )
    outs = [res.results[i]["out"][0] for i in range(NCORES)]
    full = np.concatenate(outs, axis=0).reshape(-1, 1, 1, 1).astype(np.float32)
    return full, res


def kernel(x, target, W, b):
    full, _ = run(x, target, W, b, trace=False)
    return full


# revision 9
# speedup vs baseline: 3.0268x; 1.1206x over previous
"""Trainium2 Bass kernel for nn_LocalClassifier (moe_routing).

Computation (reference):
    xr     = x.reshape(B, P, F)            # [32, 784, 2048] fp32
    Wg     = W[target]                     # [32, 2048]  per-batch gathered row
    logits = einsum('bpf,bf->bp', xr, Wg) + b[target][:, None]
    out    = sigmoid(logits).reshape(-1, 1, 1, 1)    # [25088, 1, 1, 1]

Strategy (8 NeuronCores, data parallel over B):
  - Host gathers the 4 W rows / bias values each core needs (the "routing"),
    shards B across the 8 cores (4 batches -> 3136 of the 25088 rows each),
    and pre-transposes each core's x shard to feature-major fp16 layout
    [batch, chunk-group, partition=f%128, (chunk, pixel)] so the
    TensorEngine contracts over features (K on partitions) with plain
    contiguous DMAs.  fp16 operands: PE streams single-pass (fp32 lowers
    to a 2x LO/HI pass) and HBM traffic halves; PSUM accumulates fp32.
  - The 4 batches map to the PE array's four 32-wide column groups
    (tile_position via psum base partition 32*b), so their 128x1x392
    matmuls run concurrently instead of back-to-back.
  - Streaming: 16 DMAs of 0.8 MB (4 chunk-groups x 4 batches), alternated
    across the two HWDGE rings (SP/ACT); all tiles resident (bufs=16) so
    DMA never stalls on buffer recycling.
  - Epilogue: fused bias+sigmoid on ScalarE from PSUM strips, one strided
    DMA writes the [4, 784] fp32 outputs.
  - Memory-bound: 12.8 MB/core HBM reads at ~360 GB/s -> ~36 us roofline.
"""

import sys

sys.path.insert(0, "/opt/trn_rl_repo")

import numpy as np

import concourse.bacc as bacc
import concourse.mybir as mybir
import concourse.tile as tile
from concourse.bass_utils import run_bass_kernel_spmd

B = 32      # batches
P = 784     # pixels per batch
F = 2048    # features
NCORES = 8
BPC = B // NCORES          # 4 batches per core
KC = F // 128              # 16 feature chunks of 128
KG = 4                     # chunk groups (DMA granularity)
CPG = KC // KG             # 4 chunks per group
NH = 2                     # split P into 2 matmul halves (PSUM bank = 512 fp32)
NHALF = P // NH            # 392

FP32 = mybir.dt.float32
FP16 = mybir.dt.float16

_NC_CACHE = {}


def _build_nc():
    nc = bacc.Bacc()
    xt = nc.declare_dram_parameter("xt", [BPC, KG, 128, CPG * P], FP16, isOutput=False)
    wg = nc.declare_dram_parameter("wg", [128, BPC * KC], FP16, isOutput=False)
    bg = nc.declare_dram_parameter("bg", [128, 1], FP32, isOutput=False)
    out = nc.declare_dram_parameter("out", [BPC, P], FP32, isOutput=True)

    with tile.TileContext(nc) as tc:
        with (
            tc.tile_pool(name="xpool", bufs=KG * BPC) as xpool,
            tc.tile_pool(name="cpool", bufs=1) as cpool,
            tc.tile_pool(name="psum", bufs=1, space="PSUM") as pp,
        ):
            # constants ride the ACT HWDGE ring so they don't delay x tiles
            wg_sb = cpool.tile([128, BPC * KC], FP16)
            nc.scalar.dma_start(out=wg_sb[:], in_=wg[:])
            bg_sb = cpool.tile([128, 1], FP32)
            nc.scalar.dma_start(out=bg_sb[:], in_=bg[:])
            out_sb = cpool.tile([128, P], FP32)

            # batch b accumulates in PSUM partition strip [32b, 32b+1)
            ps = [
                pp.tile([128, NHALF], FP32, name=f"ps{h}", tag=f"ps{h}")
                for h in range(NH)
            ]

            tiles = {}
            for g in range(KG):
                for b in range(BPC):
                    t = xpool.tile([128, CPG * P], FP16, name=f"x{g}{b}", tag="xt")
                    eng = nc.sync if (g * BPC + b) % 2 == 0 else nc.scalar
                    eng.dma_start(out=t[:], in_=xt[b, g])
                    tiles[g, b] = t
                for c in range(CPG):
                    k = g * CPG + c
                    for b in range(BPC):
                        col = b * KC + k
                        for h in range(NH):
                            nc.tensor.matmul(
                                ps[h][32 * b : 32 * b + 1, :],
                                wg_sb[:, col : col + 1],
                                tiles[g, b][
                                    :, c * P + h * NHALF : c * P + (h + 1) * NHALF
                                ],
                                start=(k == 0),
                                stop=(k == KC - 1),
                                tile_position=(0, 32 * b),
                            )

            for b in range(BPC):
                for h in range(NH):
                    nc.scalar.activation(
                        out_sb[32 * b : 32 * b + 1, h * NHALF : (h + 1) * NHALF],
                        ps[h][32 * b : 32 * b + 1, :],
                        mybir.ActivationFunctionType.Sigmoid,
                        bias=bg_sb[32 * b : 32 * b + 1, 0:1],
                        scale=1.0,
                    )

            nc.sync.dma_start(out=out[:], in_=out_sb[0:128:32, :])

    nc.finalize()
    return nc


def _get_nc():
    if "nc" not in _NC_CACHE:
        _NC_CACHE["nc"] = _build_nc()
    return _NC_CACHE["nc"]


def _make_in_maps(x, target, W, b):
    x = np.asarray(x, dtype=np.float32).reshape(B, P, F)
    target = np.asarray(target).astype(np.int64)
    W = np.asarray(W, dtype=np.float32)
    b = np.asarray(b, dtype=np.float32)

    Wg = W[target]          # [B, F]
    bg = b[target]          # [B]

    in_maps = []
    for m in range(NCORES):
        sl = slice(m * BPC, (m + 1) * BPC)
        # [BPC, P, KG, CPG, 128] -> [BPC, KG, 128(part), CPG, P]
        xs = x[sl].reshape(BPC, P, KG, CPG, 128).transpose(0, 2, 4, 3, 1)
        xtc = np.ascontiguousarray(xs, dtype=np.float16).reshape(
            BPC, KG, 128, CPG * P
        )
        # wg[p, b*KC + k] = Wg[b, k*128 + p]
        wgc = (
            Wg[sl]
            .reshape(BPC, KC, 128)
            .transpose(2, 0, 1)
            .reshape(128, BPC * KC)
            .astype(np.float16)
        )
        bgs = np.zeros((128, 1), np.float32)
        bgs[np.arange(BPC) * 32, 0] = bg[sl]
        in_maps.append({"xt": xtc, "wg": np.ascontiguousarray(wgc), "bg": bgs})
    return in_maps


def run(x, target, W, b, trace=False, **trace_kwargs):
    """Run on 8 cores; returns (full_output, BassKernelResults)."""
    nc = _get_nc()
    in_maps = _make_in_maps(x, target, W, b)
    res = run_bass_kernel_spmd(
        nc, in_maps, list(range(NCORES)), trace=trace, **trace_kwargs
    )
    outs = [res.results[i]["out"].reshape(-1) for i in range(NCORES)]
    full = np.concatenate(outs, axis=0).reshape(-1, 1, 1, 1).astype(np.float32)
    return full, res


def kernel(x, target, W, b):
    full, _ = run(x, target, W, b, trace=False)
    return full


# revision 10
# speedup vs baseline: 3.1655x; 1.0459x over previous
"""Trainium2 Bass kernel for nn_LocalClassifier (moe_routing).

Computation (reference):
    xr     = x.reshape(B, P, F)            # [32, 784, 2048] fp32
    Wg     = W[target]                     # [32, 2048]  per-batch gathered row
    logits = einsum('bpf,bf->bp', xr, Wg) + b[target][:, None]
    out    = sigmoid(logits).reshape(-1, 1, 1, 1)    # [25088, 1, 1, 1]

Strategy (8 NeuronCores, data parallel over B):
  - Host gathers the 4 W rows / bias values each core needs (the "routing"),
    shards B across the 8 cores (4 batches -> 3136 of the 25088 rows each),
    and pre-transposes each core's x shard to feature-major fp16 layout so
    the TensorEngine contracts over features (K on partitions) with plain
    contiguous DMAs.  fp16 operands: PE streams single-pass (fp32 lowers to
    a 2x LO/HI pass) and HBM traffic halves; PSUM accumulates fp32.
  - The 4 batches map to the PE array's four 32-wide column groups
    (tile_position (0, 32*b)), so their 128x1x392 matmuls run concurrently
    instead of back-to-back.
  - Streaming: uneven chunk-groups [5,5,5,1] x 4 batches, one DMA each
    (1.2/0.25 MB), alternated across the two HWDGE rings (SP/ACT); all
    tiles resident (bufs=16) so DMA never stalls on buffer recycling.  The
    tiny last group keeps the post-stream PE tail under 1 us.
  - Epilogue: two fused bias+sigmoid activations over PSUM partitions 0-96
    (only rows {0,32,64,96} are consumed), one strided DMA writes the
    [4, 784] fp32 outputs.
  - Memory-bound: 12.8 MB/core HBM reads at ~360 GB/s -> ~36 us roofline.
"""

import sys

sys.path.insert(0, "/opt/trn_rl_repo")

import numpy as np

import concourse.bacc as bacc
import concourse.mybir as mybir
import concourse.tile as tile
from concourse.bass_utils import run_bass_kernel_spmd

B = 32      # batches
P = 784     # pixels per batch
F = 2048    # features
NCORES = 8
BPC = B // NCORES          # 4 batches per core
KC = F // 128              # 16 feature chunks of 128
GROUPS = [5, 5, 5, 1]      # chunks per DMA group (small tail group)
NH = 2                     # split P into 2 matmul halves (PSUM bank = 512 fp32)
NHALF = P // NH            # 392

FP32 = mybir.dt.float32
FP16 = mybir.dt.float16

_NC_CACHE = {}


def _build_nc():
    nc = bacc.Bacc()
    xt = nc.declare_dram_parameter("xt", [BPC, KC * 128 * P], FP16, isOutput=False)
    wg = nc.declare_dram_parameter("wg", [128, BPC * KC], FP16, isOutput=False)
    bg = nc.declare_dram_parameter("bg", [128, 1], FP32, isOutput=False)
    out = nc.declare_dram_parameter("out", [BPC, P], FP32, isOutput=True)

    with tile.TileContext(nc) as tc:
        with (
            tc.tile_pool(name="xpool", bufs=len(GROUPS) * BPC) as xpool,
            tc.tile_pool(name="cpool", bufs=1) as cpool,
            tc.tile_pool(name="psum", bufs=1, space="PSUM") as pp,
        ):
            # constants ride the ACT HWDGE ring so they don't delay x tiles
            wg_sb = cpool.tile([128, BPC * KC], FP16)
            nc.scalar.dma_start(out=wg_sb[:], in_=wg[:])
            bg_sb = cpool.tile([128, 1], FP32)
            nc.scalar.dma_start(out=bg_sb[:], in_=bg[:])
            out_sb = cpool.tile([128, P], FP32)

            # batch b accumulates in PSUM partition strip [32b, 32b+1)
            ps = [
                pp.tile([128, NHALF], FP32, name=f"ps{h}", tag=f"ps{h}")
                for h in range(NH)
            ]

            off = 0
            di = 0
            for g, n in enumerate(GROUPS):
                tiles = []
                for b in range(BPC):
                    t = xpool.tile([128, n * P], FP16, name=f"x{g}{b}", tag="xt")
                    eng = nc.sync if di % 2 == 0 else nc.scalar
                    di += 1
                    eng.dma_start(
                        out=t[:],
                        in_=xt[b, off * 128 * P : (off + n) * 128 * P].rearrange(
                            "(p f) -> p f", p=128
                        ),
                    )
                    tiles.append(t)
                for c in range(n):
                    k = off + c
                    for b in range(BPC):
                        col = b * KC + k
                        for h in range(NH):
                            nc.tensor.matmul(
                                ps[h][32 * b : 32 * b + 1, :],
                                wg_sb[:, col : col + 1],
                                tiles[b][
                                    :, c * P + h * NHALF : c * P + (h + 1) * NHALF
                                ],
                                start=(k == 0),
                                stop=(k == KC - 1),
                                tile_position=(0, 32 * b),
                            )
                off += n

            # one activation per half over partitions 0..96; lanes other
            # than {0,32,64,96} compute on garbage and are never read
            for h in range(NH):
                nc.scalar.activation(
                    out_sb[0:97, h * NHALF : (h + 1) * NHALF],
                    ps[h][0:97, :],
                    mybir.ActivationFunctionType.Sigmoid,
                    bias=bg_sb[0:97, 0:1],
                    scale=1.0,
                )

            nc.sync.dma_start(out=out[:], in_=out_sb[0:128:32, :])

    nc.finalize()
    return nc


def _get_nc():
    if "nc" not in _NC_CACHE:
        _NC_CACHE["nc"] = _build_nc()
    return _NC_CACHE["nc"]


def _make_in_maps(x, target, W, b):
    x = np.asarray(x, dtype=np.float32).reshape(B, P, F)
    target = np.asarray(target).astype(np.int64)
    W = np.asarray(W, dtype=np.float32)
    b = np.asarray(b, dtype=np.float32)

    Wg = W[target]          # [B, F]
    bg = b[target]          # [B]

    in_maps = []
    for m in range(NCORES):
        sl = slice(m * BPC, (m + 1) * BPC)
        # (b, e, k, p) -> (b, k, p, e), fp16
        xs = (
            x[sl]
            .reshape(BPC, P, KC, 128)
            .transpose(0, 2, 3, 1)
            .astype(np.float16)
        )  # [BPC, KC, 128, P]
        # per group: (k, p, e) -> (p, k, e) so each partition's group data
        # is one contiguous run
        parts = []
        off = 0
        for n in GROUPS:
            grp = xs[:, off : off + n].transpose(0, 2, 1, 3)  # [BPC, 128, n, P]
            parts.append(grp.reshape(BPC, n * 128 * P))
            off += n
        xtc = np.ascontiguousarray(np.concatenate(parts, axis=1))
        # wg[p, b*KC + k] = Wg[b, k*128 + p]
        wgc = (
            Wg[sl]
            .reshape(BPC, KC, 128)
            .transpose(2, 0, 1)
            .reshape(128, BPC * KC)
            .astype(np.float16)
        )
        bgs = np.zeros((128, 1), np.float32)
        bgs[np.arange(BPC) * 32, 0] = bg[sl]
        in_maps.append({"xt": xtc, "wg": np.ascontiguousarray(wgc), "bg": bgs})
    return in_maps


def run(x, target, W, b, trace=False, **trace_kwargs):
    """Run on 8 cores; returns (full_output, BassKernelResults)."""
    nc = _get_nc()
    in_maps = _make_in_maps(x, target, W, b)
    res = run_bass_kernel_spmd(
        nc, in_maps, list(range(NCORES)), trace=trace, **trace_kwargs
    )
    outs = [res.results[i]["out"].reshape(-1) for i in range(NCORES)]
    full = np.concatenate(outs, axis=0).reshape(-1, 1, 1, 1).astype(np.float32)
    return full, res


def kernel(x, target, W, b):
    full, _ = run(x, target, W, b, trace=False)
    return full
